# revision 1
# baseline (speedup 1.0000x reference)
"""Trainium2 Bass kernel: batched multi-head attention.

out[b,h] = softmax(Q[b,h] @ K[b,h].T / sqrt(D)) @ V[b,h]
with B=4, H=16, S=2048, D=64, fp32.

Sharding: the 64 (b,h) pairs are split across 8 NeuronCores, 8 pairs per
core; attention is independent per pair, so no cross-core communication.

Device dataflow per pair (all matmuls in float32r — full-rate fp32):
  1. Host pre-lays inputs:
       qt  [128, 2048]: Q^T (d on partitions) duplicated into partitions
                        64..127 so two K=64-contraction matmuls can run
                        concurrently via PE row-tiling.
       kt  [128, 1024]: K^T k-tiles interleaved — k-tile 2t at partitions
                        0..63, k-tile 2t+1 at partitions 64..127, both at
                        columns [128t, 128t+128).
       vo  [128, 1040]: 16 chunks of [V_ktile | ones] of width 65 — the
                        ones column makes the PV matmul also produce the
                        softmax denominator (sum_k exp) for free.
  2. scores^T[k,q] = K^T.T @ Q^T, one [128, 512] slice per matmul
     (k-tile x 512-wide q-chunk); consecutive k-tiles alternate PE
     row-tiling strips so adjacent matmuls overlap on hardware.
  3. P^T = exp(scores^T / 8) on the scalar engine (scale folded into the
     activation's free affine). No max-subtraction: scores/8 ~ N(0,1),
     |max| < ~6, exp is safe in fp32.
  4. out65[d|sum, q] += [V|1].T @ P^T accumulated over k-tiles in PSUM.
  5. out65 -> SBUF -> HBM; host divides rows 0..63 by row 64 and
     transposes back to [q, d].

Schedule: the scalar engine (exp) is the bottleneck (~218.5us/core
floor at 128 lanes x 1.2GHz), so everything is organized to keep it
saturated with maximal-width ops. The whole core's scores stream is
chunked into [128, 1536] exp ops (3 PSUM banks, double-buffered; the
PV accumulators take the last 2 of 8 banks) taken from the GLOBAL
slice stream so chunks straddle q-chunk/pair boundaries — every exp op
is full width, amortizing the ~222-cycle per-op access overhead. A
one-chunk software pipeline (emit chunk c's score matmuls, then chunk
c-1's PV matmuls) keeps the in-order PE one exp ahead of ACT. Input
DMAs are split across the SP HWDGE ring and SWDGE (gpsimd), each
ordered by first-need, because same-ring DMAs serialize.

CoreSim cost model: 258us/core e2e, ACT 99% saturated between first
and last exp. Measured HW relative error vs fp32 reference: 1.9e-4.
"""

import sys

sys.path.insert(0, "/opt/trn_rl_repo")

import numpy as np

import concourse.bacc as bacc
import concourse.bass as bass
import concourse.mybir as mybir
from concourse.bass_utils import run_bass_kernel_spmd
from concourse.tile import TileContext

B, H, S, D = 4, 16, 2048, 64
N_CORES = 8
PAIRS = B * H              # 64 independent (b, h) attention problems
PPC = PAIRS // N_CORES     # 8 pairs per core
KT = S // 128              # 16 k-tiles of 128 rows
QC = 512                   # q-chunk width (4 chunks of 512)
CW = 1536                  # exp chunk width (3 score slices of 512)
F32 = mybir.dt.float32
F32R = mybir.dt.float32r
EXP = mybir.ActivationFunctionType.Exp
SCALE = 1.0 / np.sqrt(D)   # folded into the activation


def build_bass():
    nc = bacc.Bacc()
    qt_d = nc.declare_dram_parameter("qt", [PPC, 128, S], F32R, isOutput=False)
    kt_d = nc.declare_dram_parameter("kt", [PPC, 128, S // 2], F32R, isOutput=False)
    vo_d = nc.declare_dram_parameter("vo", [PPC, 128, KT * 65], F32R, isOutput=False)
    out_d = nc.declare_dram_parameter("ot", [PPC, 65, S], F32, isOutput=True)

    with TileContext(nc) as tc:
        with (
            tc.tile_pool(name="qt", bufs=2) as qt_pool,
            tc.tile_pool(name="kt", bufs=2) as kt_pool,
            tc.tile_pool(name="vo", bufs=2) as vo_pool,
            tc.tile_pool(name="pt", bufs=4) as pt_pool,
            tc.tile_pool(name="ob", bufs=2) as ob_pool,
            tc.tile_pool(name="ps_s", bufs=2, space="PSUM") as ps_s_pool,
            tc.tile_pool(name="ps_o", bufs=2, space="PSUM") as ps_o_pool,
        ):
            # The whole core's work is one stream of 512-wide score
            # slices (pair-major, then qc, then k-tile). Exp chunks are
            # groups of 3 consecutive slices ([128, 1536] = 3 PSUM banks)
            # taken from the GLOBAL stream — chunks freely straddle qc
            # and pair boundaries, so every exp op is full width (one
            # runt at the very end of the stream). Software-pipelined by
            # one chunk: emit chunk c's scores matmuls, then chunk c-1's
            # PV matmuls, so ACT (the bottleneck) never stalls.
            # k-tile t lives at partition strip (t%2)*64 of kt_paired,
            # column (t//2)*128; consecutive t alternate strips, so
            # adjacent scores matmuls row-tile onto different strips.
            stream = [
                (p, qc, t)
                for p in range(PPC)
                for qc in range(S // QC)
                for t in range(KT)
            ]
            # Runt chunk first: the stream's first exp only gates on two
            # matmuls (and a smaller first DMA); all later ops full-width.
            nsl = CW // 512
            runt = len(stream) % nsl or nsl
            chunks = [stream[0:runt]] + [
                stream[i : i + nsl] for i in range(runt, len(stream), nsl)
            ]
            tiles = {}   # pair -> (qt, kt, vo, ob)
            o65s = {}    # (pair, qc) -> psum accumulator
            pts = {}     # chunk idx -> pt tile

            def emit_pv(ci):
                pt = pts.pop(ci)
                for i, (p, qc, t) in enumerate(chunks[ci]):
                    o65 = o65s[(p, qc)]
                    vo, ob = tiles[p][2], tiles[p][3]
                    nc.tensor.matmul(
                        o65[:],
                        vo[:, t * 65 : (t + 1) * 65],
                        pt[:, i * 512 : (i + 1) * 512],
                        start=(t == 0),
                        stop=(t == KT - 1),
                    )
                    if t == KT - 1:
                        nc.vector.tensor_copy(
                            out=ob[:, qc * QC : (qc + 1) * QC], in_=o65[:]
                        )
                        del o65s[(p, qc)]
                        # Write each q-chunk out as soon as it's drained so
                        # the kernel tail only carries the final DMA.
                        nc.sync.dma_start(
                            out=out_d[p][:, qc * QC : (qc + 1) * QC],
                            in_=ob[:, qc * QC : (qc + 1) * QC],
                        )

            for ci, chunk in enumerate(chunks):
                w = len(chunk) * 512
                sc = ps_s_pool.tile([128, CW], F32, tag="s")
                for i, (p, qc, t) in enumerate(chunk):
                    if p not in tiles:
                        # Stage DMAs so the first scores matmul's operands
                        # (kt cols 0:128, qt cols 0:512) land first.
                        # Two DMA issue paths in parallel, each ordered by
                        # when the data is first needed: the SP HWDGE ring
                        # carries the scores-critical pieces, SWDGE
                        # (gpsimd/Pool — otherwise idle) carries vo (needed
                        # by the first PV) and the bulk remainders.
                        kt = kt_pool.tile([128, S // 2], F32R)
                        nc.sync.dma_start(
                            out=kt[:, 0:256], in_=kt_d[p][:, 0:256]
                        )
                        qt = qt_pool.tile([128, S], F32R)
                        nc.gpsimd.dma_start(out=qt[:, 0:512], in_=qt_d[p][:, 0:512])
                        vo = vo_pool.tile([128, KT * 65], F32R)
                        nc.gpsimd.dma_start(out=vo[:], in_=vo_d[p])
                        nc.gpsimd.dma_start(
                            out=kt[:, 256 : S // 2], in_=kt_d[p][:, 256 : S // 2]
                        )
                        nc.sync.dma_start(
                            out=qt[:, 512:1024], in_=qt_d[p][:, 512:1024]
                        )
                        nc.gpsimd.dma_start(out=qt[:, 1024:S], in_=qt_d[p][:, 1024:S])
                        ob = ob_pool.tile([65, S], F32)
                        tiles[p] = (qt, kt, vo, ob)
                    qt, kt = tiles[p][0], tiles[p][1]
                    if (p, qc) not in o65s:
                        o65s[(p, qc)] = ps_o_pool.tile(
                            [65, QC], F32, name="o65", tag="o65"
                        )
                    strip = (t % 2) * 64
                    col = (t // 2) * 128
                    nc.tensor.matmul(
                        sc[:, i * 512 : (i + 1) * 512],
                        kt[strip : strip + 64, col : col + 128],
                        qt[strip : strip + 64, qc * QC : (qc + 1) * QC],
                        start=True,
                        stop=True,
                        tile_position=(strip, 0),
                    )
                pt = pt_pool.tile([128, CW], F32R, tag="p")
                nc.scalar.activation(pt[:, :w], sc[:, :w], EXP, scale=SCALE)
                pts[ci] = pt
                if ci > 0:
                    emit_pv(ci - 1)
            emit_pv(len(chunks) - 1)
    nc.compile()
    return nc


def _prep_inputs(query, key, value):
    """Host-side layout prep. Returns per-core input maps."""
    q = np.ascontiguousarray(query.reshape(PAIRS, S, D))
    k = np.ascontiguousarray(key.reshape(PAIRS, S, D))
    v = np.ascontiguousarray(value.reshape(PAIRS, S, D))

    qt = q.transpose(0, 2, 1)                     # [PAIRS, 64, 2048]
    qt_dup = np.concatenate([qt, qt], axis=1)     # [PAIRS, 128, 2048]
    qt_dup = np.ascontiguousarray(qt_dup, dtype=np.float32)

    # kt_paired[p, 0:64, 128t+j]  = K^T[p, :, 256t + j]
    # kt_paired[p, 64:128, 128t+j] = K^T[p, :, 256t + 128 + j]
    kt = k.transpose(0, 2, 1).reshape(PAIRS, D, KT // 2, 2, 128)
    kt_paired = np.ascontiguousarray(
        kt.transpose(0, 3, 1, 2, 4).reshape(PAIRS, 128, S // 2), dtype=np.float32
    )

    vt = v.reshape(PAIRS, KT, 128, D).transpose(0, 2, 1, 3)  # [PAIRS,128,KT,64]
    vo = np.empty((PAIRS, 128, KT, 65), dtype=np.float32)
    vo[:, :, :, :D] = vt
    vo[:, :, :, D] = 1.0
    vo = vo.reshape(PAIRS, 128, KT * 65)

    in_maps = []
    for c in range(N_CORES):
        sl = slice(c * PPC, (c + 1) * PPC)
        in_maps.append(
            {
                "qt": np.ascontiguousarray(qt_dup[sl]),
                "kt": np.ascontiguousarray(kt_paired[sl]),
                "vo": np.ascontiguousarray(vo[sl]),
            }
        )
    return in_maps


_CACHED_NC = None


def kernel(query, key, value, _want_results_obj=False, _trace=False):
    global _CACHED_NC
    if _CACHED_NC is None:
        _CACHED_NC = build_bass()
    nc = _CACHED_NC

    in_maps = _prep_inputs(query, key, value)
    res = run_bass_kernel_spmd(
        nc, in_maps, core_ids=list(range(N_CORES)), trace=_trace
    )

    ot = np.concatenate([res.results[c]["ot"] for c in range(N_CORES)], axis=0)
    out = ot[:, :D, :] / ot[:, D : D + 1, :]     # normalize by softmax denom
    out = out.transpose(0, 2, 1).reshape(B, H, S, D).astype(np.float32)
    if _want_results_obj:
        return out, res
    return out


if __name__ == "__main__":
    rng = np.random.default_rng(0)
    q = rng.standard_normal((B, H, S, D), dtype=np.float32)
    k = rng.standard_normal((B, H, S, D), dtype=np.float32)
    v = rng.standard_normal((B, H, S, D), dtype=np.float32)
    o = kernel(query=q, key=k, value=v)
    print("out shape:", o.shape, o.dtype)



# revision 6
# speedup vs baseline: 1.3532x; 1.3532x over previous
"""Trainium2 Bass kernel: batched multi-head attention.

out[b,h] = softmax(Q[b,h] @ K[b,h].T / sqrt(D)) @ V[b,h]
with B=4, H=16, S=2048, D=64, fp32 in/out.

Sharding: the 64 (b,h) pairs are split across 8 NeuronCores, 8 pairs per
core; attention is independent per pair, so no cross-core communication.

Design (v2) — dual-engine softmax + full-utilization PV:

  Under the CoreSim cost model a matmul costs out_free_cols x 0.4167ns
  regardless of K/M, and exp on the scalar engine costs 0.833ns/elem/lane.
  The baseline was ACT-bound (~250us busy). This version:

  1. PV reshape: out[q=128, 65] = pt[k=128, q=128].T @ [V|1][k=128, 65]
     per (q-tile, k-tile) in bf16 (1 cycle/row at any N). PV drops from
     109us to 55.5us; PE total = scores 109.2 + PV 55.5 = 165us busy.
     The ones column makes column 64 the softmax denominator.
  2. exp is split between ACT (exact, exp -> bf16) and DVE (Schraudolph:
     one tensor_scalar computing i16 = rint(x*(128 log2 e)/8 + B) written
     through an int16-bitcast view of the bf16 pt tile — the int16 bit
     pattern IS bf16 2^u, a ~1.8% rms approximation of exp). Greedy
     interleave keeps both engines at ~149us < PE.
  3. Scores stay fp32r (exact): sc[k=128, q=512] = K^T-tile.T @ Q^T-chunk,
     two 512-slices per PSUM chunk [128,1024]; pipeline: scores(c) ->
     exp(c) -> PV(c-2), sc triple-buffered (6 banks) + 2 accumulator
     banks (4 sub-bank [128,65] slots each) = 8 PSUM banks exactly.
  4. Q/K/V are pre-laid host-side in bf16 (halves DMA); inputs ride the
     SP HWDGE ring (qt, kt) and SWDGE (vt); outputs [128,260] drain
     PSUM->SBUF on DVE and DMA out per (pair, qc), alternating SP/SWDGE.
     Output rows are (qc, j, q) interleaved; host undoes it and divides
     by the denominator column.

CoreSim cost model: ~170us/core e2e (PE-bound). Measured HW (PJRT)
relative error vs fp32 reference: ~1.2e-2 (Schraudolph share ~0.42).
"""

import sys

sys.path.insert(0, "/opt/trn_rl_repo")

import numpy as np
import ml_dtypes

import concourse.bacc as bacc
import concourse.mybir as mybir
from concourse.bass_utils import run_bass_kernel_spmd
from concourse.tile import TileContext

B, H, S, D = 4, 16, 2048, 64
N_CORES = 8
PAIRS = B * H              # 64 independent (b, h) attention problems
PPC = PAIRS // N_CORES     # 8 pairs per core
KT = S // 128              # 16 k-tiles of 128 rows
NQC = 4                    # 4 q-chunks of 512
CPQ = 8                    # chunks per (pair, qc): [128, 1024] = 2 score slices
F32 = mybir.dt.float32
F32R = mybir.dt.float32r
BF16 = mybir.dt.bfloat16
I16 = mybir.dt.int16
EXP = mybir.ActivationFunctionType.Exp
MULT = mybir.AluOpType.mult
ADD = mybir.AluOpType.add
SCALE = 1.0 / np.sqrt(D)

# Schraudolph bf16 exp2 constants (round-to-nearest f32->i16 on DVE):
# i16 = rint(x*SCALE * 128/ln2 + (127*128 - 7.35)); bitcast i16 -> bf16
# approximates exp(x*SCALE) with ~1.78% rms, ~0 mean error.
A_EXP = float(128.0 / np.log(2.0)) * SCALE
B_EXP = 127.0 * 128.0 - 7.35

# Cost-model constants used only to balance the ACT/DVE greedy split.
ACT_CHUNK_NS = 1024 * 0.8333 + 185
DVE_CHUNK_NS = 1024 * 1.0417 + 125
DVE_DRAIN_NS = 260 * 1.0417 + 125
ACT_TABLE_NS = 1283


def build_bass():
    nc = bacc.Bacc()
    qt_d = nc.declare_dram_parameter("qt", [PPC, 64, S], BF16, isOutput=False)
    kt_d = nc.declare_dram_parameter("kt", [PPC, 64, S], BF16, isOutput=False)
    vt_d = nc.declare_dram_parameter("vt", [PPC, 128, KT * 65], BF16, isOutput=False)
    # output rows interleaved (qc, q, j): host reorders + divides by col 64
    ot_d = nc.declare_dram_parameter("ot", [PPC, NQC, 128, 4, 65], F32, isOutput=True)

    with TileContext(nc) as tc:
        with (
            tc.tile_pool(name="qt", bufs=2) as qt_pool,
            tc.tile_pool(name="kt", bufs=2) as kt_pool,
            tc.tile_pool(name="vt", bufs=2) as vt_pool,
            tc.tile_pool(name="pt", bufs=4) as pt_pool,
            tc.tile_pool(name="ob", bufs=2) as ob_pool,
            tc.tile_pool(name="ps_s", bufs=3, space="PSUM") as ps_s_pool,
            tc.tile_pool(name="ps_a", bufs=2, space="PSUM") as ps_a_pool,
        ):
            chunks = [
                (p, qc, u) for p in range(PPC) for qc in range(NQC) for u in range(CPQ)
            ]
            tiles = {}    # pair -> (qt, kt, vt, ob)
            accs = {}     # (pair, qc) -> [128, 512] psum accumulator (4 slots)
            pts = {}      # chunk idx -> pt tile
            act_t, dve_t = ACT_TABLE_NS, 0.0
            out_ring = [nc.sync, nc.gpsimd]

            def load_pair(p):
                if p in tiles or p >= PPC:
                    return
                qt = qt_pool.tile([64, S], BF16)
                kt = kt_pool.tile([64, S], BF16)
                vt = vt_pool.tile([128, KT * 65], BF16)
                nc.sync.dma_start(out=qt[:], in_=qt_d[p])
                nc.sync.dma_start(out=kt[:], in_=kt_d[p])
                nc.gpsimd.dma_start(out=vt[:], in_=vt_d[p])
                ob = ob_pool.tile([128, NQC * 260], F32)
                tiles[p] = (qt, kt, vt, ob)

            def emit_pv(ci):
                nonlocal dve_t
                p, qc, u = chunks[ci]
                pt = pts.pop(ci)
                _, _, vt, ob = tiles[p]
                acc = accs[(p, qc)]
                # start=True marks the whole 2KB PSUM zero-region (bank) as
                # pending-zero, so it must be issued exactly ONCE per bank
                # accumulation round: slots j=1..3's first writes clear their
                # own pending bytes (replace), later writes accumulate.
                for v in range(2):
                    t = 2 * u + v
                    for j in range(4):
                        nc.tensor.matmul(
                            acc[:, j * 65 : (j + 1) * 65],
                            pt[:, v * 512 + j * 128 : v * 512 + (j + 1) * 128],
                            vt[:, t * 65 : (t + 1) * 65],
                            start=(t == 0 and j == 0),
                            stop=(t == KT - 1 and j == 3),
                            skip_group_check=True,
                        )
                if u == CPQ - 1:
                    # drain the 4 accumulated [128,65] slots and write out
                    nc.vector.tensor_copy(
                        out=ob[:, qc * 260 : (qc + 1) * 260], in_=acc[:, 0:260]
                    )
                    dve_t += DVE_DRAIN_NS
                    del accs[(p, qc)]
                    out_ring[qc % 2].dma_start(
                        out=ot_d[p][qc], in_=ob[:, qc * 260 : (qc + 1) * 260]
                    )

            for ci, (p, qc, u) in enumerate(chunks):
                if qc == 0 and u == 0:
                    load_pair(p)
                    load_pair(p + 1)  # prefetch next pair during this one
                qt, kt, vt, ob = tiles[p]
                if u == 0:
                    accs[(p, qc)] = ps_a_pool.tile(
                        [128, 512], F32, name="acc", tag="acc"
                    )
                sc = ps_s_pool.tile([128, 1024], F32, tag="s")
                for v in range(2):
                    t = 2 * u + v
                    nc.tensor.matmul(
                        sc[:, v * 512 : (v + 1) * 512],
                        kt[:, t * 128 : (t + 1) * 128],
                        qt[:, qc * 512 : (qc + 1) * 512],
                        start=True,
                        stop=True,
                    )
                pt = pt_pool.tile([128, 1024], BF16, tag="p")
                if act_t + ACT_CHUNK_NS <= dve_t + DVE_CHUNK_NS:
                    nc.scalar.activation(pt[:], sc[:], EXP, scale=SCALE)
                    act_t += ACT_CHUNK_NS
                else:
                    nc.vector.tensor_scalar(
                        out=pt[:].bitcast(I16),
                        in0=sc[:],
                        scalar1=A_EXP,
                        scalar2=B_EXP,
                        op0=MULT,
                        op1=ADD,
                    )
                    dve_t += DVE_CHUNK_NS
                pts[ci] = pt
                if ci >= 2:
                    emit_pv(ci - 2)
            emit_pv(len(chunks) - 2)
            emit_pv(len(chunks) - 1)
    nc.compile()
    return nc


def _prep_inputs(query, key, value):
    """Host-side layout prep. Returns per-core input maps."""
    q = query.reshape(PAIRS, S, D)
    k = key.reshape(PAIRS, S, D)
    v = value.reshape(PAIRS, S, D)

    qt = np.ascontiguousarray(q.transpose(0, 2, 1)).astype(ml_dtypes.bfloat16)
    kt = np.ascontiguousarray(k.transpose(0, 2, 1)).astype(ml_dtypes.bfloat16)

    vt = v.reshape(PAIRS, KT, 128, D).transpose(0, 2, 1, 3)  # [PAIRS,128,KT,64]
    vo = np.empty((PAIRS, 128, KT, 65), dtype=ml_dtypes.bfloat16)
    vo[:, :, :, :D] = vt.astype(ml_dtypes.bfloat16)
    vo[:, :, :, D] = 1.0
    vo = vo.reshape(PAIRS, 128, KT * 65)

    in_maps = []
    for c in range(N_CORES):
        sl = slice(c * PPC, (c + 1) * PPC)
        in_maps.append(
            {
                "qt": np.ascontiguousarray(qt[sl]),
                "kt": np.ascontiguousarray(kt[sl]),
                "vt": np.ascontiguousarray(vo[sl]),
            }
        )
    return in_maps


_CACHED_NC = None


def kernel(query, key, value, _want_results_obj=False, _trace=False):
    global _CACHED_NC
    if _CACHED_NC is None:
        _CACHED_NC = build_bass()
    nc = _CACHED_NC

    in_maps = _prep_inputs(query, key, value)
    res = run_bass_kernel_spmd(
        nc, in_maps, core_ids=list(range(N_CORES)), trace=_trace
    )

    # per core: [PPC, 4, 128, 4, 65] with rows (qc, q, j) -> (qc, j, q)
    ot = np.concatenate([res.results[c]["ot"] for c in range(N_CORES)], axis=0)
    ot = ot.transpose(0, 1, 3, 2, 4).reshape(PAIRS, S, 65)
    out = ot[:, :, :D] / ot[:, :, D : D + 1]
    out = out.reshape(B, H, S, D).astype(np.float32)
    if _want_results_obj:
        return out, res
    return out


if __name__ == "__main__":
    rng = np.random.default_rng(0)
    q = rng.standard_normal((B, H, S, D), dtype=np.float32)
    k = rng.standard_normal((B, H, S, D), dtype=np.float32)
    v = rng.standard_normal((B, H, S, D), dtype=np.float32)
    o = kernel(query=q, key=k, value=v)
    print("out shape:", o.shape, o.dtype)


# revision 8
# speedup vs baseline: 1.4565x; 1.0763x over previous
"""Trainium2 Bass kernel: batched multi-head attention.

out[b,h] = softmax(Q[b,h] @ K[b,h].T / sqrt(D)) @ V[b,h]
with B=4, H=16, S=2048, D=64, fp32 in/out.

Sharding: the 64 (b,h) pairs are split across 8 NeuronCores, 8 pairs per
core; attention is independent per pair, so no cross-core communication.

Design (v2) — dual-engine softmax + full-utilization PV:

  Under the CoreSim cost model a matmul costs out_free_cols x 0.4167ns
  regardless of K/M, and exp on the scalar engine costs 0.833ns/elem/lane.
  The baseline was ACT-bound (~250us busy). This version:

  1. PV reshape: out[q=128, 65] = pt[k=128, q=128].T @ [V|1][k=128, 65]
     per (q-tile, k-tile) in bf16 (1 cycle/row at any N). PV drops from
     109us to 55.5us; PE total = scores 109.2 + PV 55.5 = 165us busy.
     The ones column makes column 64 the softmax denominator.
  2. exp is split between ACT (exact, exp -> bf16) and DVE (Schraudolph:
     one tensor_scalar computing i16 = rint(x*(128 log2 e)/8 + B) written
     through an int16-bitcast view of the bf16 pt tile — the int16 bit
     pattern IS bf16 2^u, a ~1.8% rms approximation of exp). Greedy
     interleave keeps both engines at ~149us < PE.
  3. Scores stay fp32r (exact): sc[k=128, q=512] = K^T-tile.T @ Q^T-chunk,
     two 512-slices per PSUM chunk [128,1024]; pipeline: scores(c) ->
     exp(c) -> PV(c-2), sc triple-buffered (6 banks) + 2 accumulator
     banks (4 sub-bank [128,65] slots each) = 8 PSUM banks exactly.
  4. Q/K/V are pre-laid host-side in bf16 (halves DMA); inputs ride the
     SP HWDGE ring (qt, kt) and SWDGE (vt); outputs [128,260] drain
     PSUM->SBUF on DVE and DMA out per (pair, qc), alternating SP/SWDGE.
     Output rows are (qc, j, q) interleaved; host undoes it and divides
     by the denominator column.

CoreSim cost model: ~170us/core e2e (PE-bound). Measured HW (PJRT)
relative error vs fp32 reference: ~1.2e-2 (Schraudolph share ~0.42).
"""

import sys

sys.path.insert(0, "/opt/trn_rl_repo")

import numpy as np
import ml_dtypes

import concourse.bacc as bacc
import concourse.mybir as mybir
from concourse.bass_utils import run_bass_kernel_spmd
from concourse.tile import TileContext

B, H, S, D = 4, 16, 2048, 64
N_CORES = 8
PAIRS = B * H              # 64 independent (b, h) attention problems
PPC = PAIRS // N_CORES     # 8 pairs per core
KT = S // 128              # 16 k-tiles of 128 rows
NQC = 4                    # 4 q-chunks of 512
CPQ = 8                    # chunks per (pair, qc): [128, 1024] = 2 score slices
F32 = mybir.dt.float32
F32R = mybir.dt.float32r
BF16 = mybir.dt.bfloat16
I16 = mybir.dt.int16
EXP = mybir.ActivationFunctionType.Exp
MULT = mybir.AluOpType.mult
ADD = mybir.AluOpType.add
SCALE = 1.0 / np.sqrt(D)

# Schraudolph bf16 exp2 constants (round-to-nearest f32->i16 on DVE):
# i16 = rint(x*SCALE * 128/ln2 + (127*128 - 7.35)); bitcast i16 -> bf16
# approximates exp(x*SCALE) with ~1.78% rms, ~0 mean error.
A_EXP = float(128.0 / np.log(2.0)) * SCALE
B_EXP = 127.0 * 128.0 - 7.35

# Cost-model constants used only to balance the ACT/DVE greedy split.
ACT_CHUNK_NS = 1024 * 0.8333 + 185
DVE_CHUNK_NS = 1024 * 1.0417 + 125
DVE_DRAIN_NS = 260 * 1.0417 + 125
ACT_TABLE_NS = 1283


def build_bass():
    nc = bacc.Bacc()
    qt_d = nc.declare_dram_parameter("qt", [PPC, 64, S], BF16, isOutput=False)
    kt_d = nc.declare_dram_parameter("kt", [PPC, 64, S], BF16, isOutput=False)
    vt_d = nc.declare_dram_parameter("vt", [PPC, 128, KT * 65], BF16, isOutput=False)
    # output rows interleaved (qc, q, j): host reorders + divides by col 64
    ot_d = nc.declare_dram_parameter("ot", [PPC, NQC, 128, 4, 65], F32, isOutput=True)

    with TileContext(nc) as tc:
        with (
            tc.tile_pool(name="qt", bufs=2) as qt_pool,
            tc.tile_pool(name="kt", bufs=2) as kt_pool,
            tc.tile_pool(name="vt", bufs=2) as vt_pool,
            tc.tile_pool(name="pt", bufs=5) as pt_pool,
            tc.tile_pool(name="ob", bufs=2) as ob_pool,
            tc.tile_pool(name="ps_s", bufs=3, space="PSUM") as ps_s_pool,
            tc.tile_pool(name="ps_a", bufs=2, space="PSUM") as ps_a_pool,
        ):
            chunks = [
                (p, qc, u) for p in range(PPC) for qc in range(NQC) for u in range(CPQ)
            ]
            tiles = {}    # pair -> (qt, kt, vt, ob)
            accs = {}     # (pair, qc) -> [128, 512] psum accumulator (4 slots)
            pts = {}      # chunk idx -> pt tile
            act_t, dve_t = ACT_TABLE_NS, 0.0
            out_ring = [nc.sync, nc.gpsimd]

            def load_pair(p):
                if p in tiles or p >= PPC:
                    return
                qt = qt_pool.tile([64, S], BF16)
                kt = kt_pool.tile([64, S], BF16)
                vt = vt_pool.tile([128, KT * 65], BF16)
                if p == 0:
                    # critical pieces first so the first scores matmul can
                    # start ~1.7us in instead of waiting for full 4KB rows
                    nc.sync.dma_start(out=kt[:, 0:256], in_=kt_d[p][:, 0:256])
                    nc.sync.dma_start(out=qt[:, 0:512], in_=qt_d[p][:, 0:512])
                    nc.gpsimd.dma_start(out=vt[:], in_=vt_d[p])
                    nc.sync.dma_start(out=kt[:, 256:S], in_=kt_d[p][:, 256:S])
                    nc.gpsimd.dma_start(out=qt[:, 512:S], in_=qt_d[p][:, 512:S])
                else:
                    nc.sync.dma_start(out=qt[:], in_=qt_d[p])
                    nc.sync.dma_start(out=kt[:], in_=kt_d[p])
                    nc.gpsimd.dma_start(out=vt[:], in_=vt_d[p])
                ob = ob_pool.tile([128, NQC * 260], F32)
                tiles[p] = (qt, kt, vt, ob)

            def emit_pv(ci):
                nonlocal dve_t
                p, qc, u = chunks[ci]
                pt = pts.pop(ci)
                _, _, vt, ob = tiles[p]
                acc = accs[(p, qc)]
                # start=True marks the whole 2KB PSUM zero-region (bank) as
                # pending-zero, so it must be issued exactly ONCE per bank
                # accumulation round: slots j=1..3's first writes clear their
                # own pending bytes (replace), later writes accumulate.
                for v in range(2):
                    t = 2 * u + v
                    for j in range(4):
                        nc.tensor.matmul(
                            acc[:, j * 65 : (j + 1) * 65],
                            pt[:, v * 512 + j * 128 : v * 512 + (j + 1) * 128],
                            vt[:, t * 65 : (t + 1) * 65],
                            start=(t == 0 and j == 0),
                            stop=(t == KT - 1 and j == 3),
                            skip_group_check=True,
                        )
                if u == CPQ - 1:
                    # drain the 4 accumulated [128,65] slots and write out
                    nc.vector.tensor_copy(
                        out=ob[:, qc * 260 : (qc + 1) * 260], in_=acc[:, 0:260]
                    )
                    dve_t += DVE_DRAIN_NS
                    del accs[(p, qc)]
                    out_ring[qc % 2].dma_start(
                        out=ot_d[p][qc], in_=ob[:, qc * 260 : (qc + 1) * 260]
                    )

            for ci, (p, qc, u) in enumerate(chunks):
                if qc == 0 and u == 0:
                    load_pair(p)
                    load_pair(p + 1)  # prefetch next pair during this one
                qt, kt, vt, ob = tiles[p]
                if u == 0:
                    accs[(p, qc)] = ps_a_pool.tile(
                        [128, 512], F32, name="acc", tag="acc"
                    )
                sc = ps_s_pool.tile([128, 1024], F32, tag="s")
                for v in range(2):
                    t = 2 * u + v
                    nc.tensor.matmul(
                        sc[:, v * 512 : (v + 1) * 512],
                        kt[:, t * 128 : (t + 1) * 128],
                        qt[:, qc * 512 : (qc + 1) * 512],
                        start=True,
                        stop=True,
                    )
                pt = pt_pool.tile([128, 1024], BF16, tag="p")
                if act_t + ACT_CHUNK_NS <= dve_t + DVE_CHUNK_NS:
                    nc.scalar.activation(pt[:], sc[:], EXP, scale=SCALE)
                    act_t += ACT_CHUNK_NS
                else:
                    nc.vector.tensor_scalar(
                        out=pt[:].bitcast(I16),
                        in0=sc[:],
                        scalar1=A_EXP,
                        scalar2=B_EXP,
                        op0=MULT,
                        op1=ADD,
                    )
                    dve_t += DVE_CHUNK_NS
                pts[ci] = pt
                if ci >= 3:
                    emit_pv(ci - 3)
            for ci in range(len(chunks) - 3, len(chunks)):
                emit_pv(ci)
    nc.compile()
    return nc


def _prep_inputs(query, key, value):
    """Host-side layout prep. Returns per-core input maps."""
    q = query.reshape(PAIRS, S, D)
    k = key.reshape(PAIRS, S, D)
    v = value.reshape(PAIRS, S, D)

    qt = np.ascontiguousarray(q.transpose(0, 2, 1)).astype(ml_dtypes.bfloat16)
    kt = np.ascontiguousarray(k.transpose(0, 2, 1)).astype(ml_dtypes.bfloat16)

    vt = v.reshape(PAIRS, KT, 128, D).transpose(0, 2, 1, 3)  # [PAIRS,128,KT,64]
    vo = np.empty((PAIRS, 128, KT, 65), dtype=ml_dtypes.bfloat16)
    vo[:, :, :, :D] = vt.astype(ml_dtypes.bfloat16)
    vo[:, :, :, D] = 1.0
    vo = vo.reshape(PAIRS, 128, KT * 65)

    in_maps = []
    for c in range(N_CORES):
        sl = slice(c * PPC, (c + 1) * PPC)
        in_maps.append(
            {
                "qt": np.ascontiguousarray(qt[sl]),
                "kt": np.ascontiguousarray(kt[sl]),
                "vt": np.ascontiguousarray(vo[sl]),
            }
        )
    return in_maps


_CACHED_NC = None


def kernel(query, key, value, _want_results_obj=False, _trace=False):
    global _CACHED_NC
    if _CACHED_NC is None:
        _CACHED_NC = build_bass()
    nc = _CACHED_NC

    in_maps = _prep_inputs(query, key, value)
    res = run_bass_kernel_spmd(
        nc, in_maps, core_ids=list(range(N_CORES)), trace=_trace
    )

    # per core: [PPC, 4, 128, 4, 65] with rows (qc, q, j) -> (qc, j, q)
    ot = np.concatenate([res.results[c]["ot"] for c in range(N_CORES)], axis=0)
    ot = ot.transpose(0, 1, 3, 2, 4).reshape(PAIRS, S, 65)
    out = ot[:, :, :D] / ot[:, :, D : D + 1]
    out = out.reshape(B, H, S, D).astype(np.float32)
    if _want_results_obj:
        return out, res
    return out


if __name__ == "__main__":
    rng = np.random.default_rng(0)
    q = rng.standard_normal((B, H, S, D), dtype=np.float32)
    k = rng.standard_normal((B, H, S, D), dtype=np.float32)
    v = rng.standard_normal((B, H, S, D), dtype=np.float32)
    o = kernel(query=q, key=k, value=v)
    print("out shape:", o.shape, o.dtype)


# revision 12
# speedup vs baseline: 1.4718x; 1.0105x over previous
"""Trainium2 Bass kernel: batched multi-head attention.

out[b,h] = softmax(Q[b,h] @ K[b,h].T / sqrt(D)) @ V[b,h]
with B=4, H=16, S=2048, D=64, fp32 in/out.

Sharding: the 64 (b,h) pairs are split across 8 NeuronCores, 8 pairs per
core; attention is independent per pair, so no cross-core communication.

Design (v2) — dual-engine softmax + full-utilization PV:

  Under the CoreSim cost model a matmul costs out_free_cols x 0.4167ns
  regardless of K/M, and exp on the scalar engine costs 0.833ns/elem/lane.
  The baseline was ACT-bound (~250us busy). This version:

  1. PV reshape: out[q=128, 65] = pt[k=128, q=128].T @ [V|1][k=128, 65]
     per (q-tile, k-tile) in bf16 (1 cycle/row at any N). PV drops from
     109us to 55.5us; PE total = scores 109.2 + PV 55.5 = 165us busy.
     The ones column makes column 64 the softmax denominator.
  2. exp is split between ACT (exact, exp -> bf16) and DVE (Schraudolph:
     one tensor_scalar computing i16 = rint(x*(128 log2 e)/8 + B) written
     through an int16-bitcast view of the bf16 pt tile — the int16 bit
     pattern IS bf16 2^u, a ~1.8% rms approximation of exp). Greedy
     interleave keeps both engines at ~149us < PE.
  3. Scores stay fp32r (exact): sc[k=128, q=512] = K^T-tile.T @ Q^T-chunk,
     two 512-slices per PSUM chunk [128,1024]; pipeline: scores(c) ->
     exp(c) -> PV(c-2), sc triple-buffered (6 banks) + 2 accumulator
     banks (4 sub-bank [128,65] slots each) = 8 PSUM banks exactly.
  4. Q/K/V are pre-laid host-side in bf16 (halves DMA); inputs ride the
     SP HWDGE ring (qt, kt) and SWDGE (vt); outputs [128,260] drain
     PSUM->SBUF on DVE and DMA out per (pair, qc), alternating SP/SWDGE.
     Output rows are (qc, j, q) interleaved; host undoes it and divides
     by the denominator column.

CoreSim cost model: ~170us/core e2e (PE-bound). Measured HW (PJRT)
relative error vs fp32 reference: ~1.2e-2 (Schraudolph share ~0.42).
"""

import sys

sys.path.insert(0, "/opt/trn_rl_repo")

import numpy as np
import ml_dtypes

import concourse.bacc as bacc
import concourse.mybir as mybir
from concourse.bass_utils import run_bass_kernel_spmd
from concourse.tile import TileContext

B, H, S, D = 4, 16, 2048, 64
N_CORES = 8
PAIRS = B * H              # 64 independent (b, h) attention problems
PPC = PAIRS // N_CORES     # 8 pairs per core
KT = S // 128              # 16 k-tiles of 128 rows
NQC = 4                    # 4 q-chunks of 512
CPQ = 8                    # chunks per (pair, qc): [128, 1024] = 2 score slices
F32 = mybir.dt.float32
F32R = mybir.dt.float32r
BF16 = mybir.dt.bfloat16
I16 = mybir.dt.int16
EXP = mybir.ActivationFunctionType.Exp
COPY = mybir.ActivationFunctionType.Copy
MULT = mybir.AluOpType.mult
ADD = mybir.AluOpType.add
SCALE = 1.0 / np.sqrt(D)

# Schraudolph bf16 exp2 constants (round-to-nearest f32->i16 on DVE):
# i16 = rint(x*SCALE * 128/ln2 + (127*128 - 7.35)); bitcast i16 -> bf16
# approximates exp(x*SCALE) with ~1.78% rms, ~0 mean error.
A_EXP = float(128.0 / np.log(2.0)) * SCALE
B_EXP = 127.0 * 128.0 - 7.35

# Cost-model constants used only to balance the ACT/DVE greedy split.
ACT_CHUNK_NS = 1024 * 0.8333 + 185
DVE_CHUNK_NS = 1024 * 1.0417 + 125
DVE_DRAIN_NS = 260 * 1.0417 + 125
ACT_DRAIN_NS = 260 * 0.8333 + 185
ACT_TABLE_NS = 1283


def build_bass():
    nc = bacc.Bacc()
    qt_d = nc.declare_dram_parameter("qt", [PPC, 64, S], BF16, isOutput=False)
    kt_d = nc.declare_dram_parameter("kt", [PPC, 64, S], BF16, isOutput=False)
    vt_d = nc.declare_dram_parameter("vt", [PPC, 128, KT * 65], BF16, isOutput=False)
    # output rows interleaved (qc, q, j): host reorders + divides by col 64
    ot_d = nc.declare_dram_parameter("ot", [PPC, NQC, 128, 4, 65], F32, isOutput=True)

    with TileContext(nc) as tc:
        with (
            tc.tile_pool(name="qt", bufs=2) as qt_pool,
            tc.tile_pool(name="kt", bufs=2) as kt_pool,
            tc.tile_pool(name="vt", bufs=2) as vt_pool,
            tc.tile_pool(name="pt", bufs=5) as pt_pool,
            tc.tile_pool(name="ob", bufs=2) as ob_pool,
            tc.tile_pool(name="ps_s", bufs=3, space="PSUM") as ps_s_pool,
            tc.tile_pool(name="ps_a", bufs=2, space="PSUM") as ps_a_pool,
        ):
            chunks = [
                (p, qc, u) for p in range(PPC) for qc in range(NQC) for u in range(CPQ)
            ]
            tiles = {}    # pair -> (qt, kt, vt, ob)
            accs = {}     # (pair, qc) -> [128, 512] psum accumulator (4 slots)
            pts = {}      # chunk idx -> pt tile
            act_t, dve_t = ACT_TABLE_NS, 0.0
            out_ring = [nc.sync, nc.gpsimd]

            def load_pair(p):
                if p in tiles or p >= PPC:
                    return
                qt = qt_pool.tile([64, S], BF16)
                kt = kt_pool.tile([64, S], BF16)
                vt = vt_pool.tile([128, KT * 65], BF16)
                if p == 0:
                    # critical pieces first, on parallel rings, so the first
                    # scores matmul starts ~1.7us in instead of ~4.5us
                    nc.sync.dma_start(out=kt[:, 0:256], in_=kt_d[p][:, 0:256])
                    nc.scalar.dma_start(out=qt[:, 0:512], in_=qt_d[p][:, 0:512])
                    nc.gpsimd.dma_start(out=vt[:], in_=vt_d[p])
                    nc.sync.dma_start(out=kt[:, 256:S], in_=kt_d[p][:, 256:S])
                    nc.gpsimd.dma_start(out=qt[:, 512:S], in_=qt_d[p][:, 512:S])
                else:
                    nc.sync.dma_start(out=qt[:], in_=qt_d[p])
                    nc.sync.dma_start(out=kt[:], in_=kt_d[p])
                    nc.gpsimd.dma_start(out=vt[:], in_=vt_d[p])
                ob = ob_pool.tile([128, NQC * 260], F32)
                tiles[p] = (qt, kt, vt, ob)

            def emit_pv(ci):
                nonlocal dve_t
                p, qc, u = chunks[ci]
                pt = pts.pop(ci)
                _, _, vt, ob = tiles[p]
                acc = accs[(p, qc)]
                # start=True marks the whole 2KB PSUM zero-region (bank) as
                # pending-zero, so it must be issued exactly ONCE per bank
                # accumulation round: slots j=1..3's first writes clear their
                # own pending bytes (replace), later writes accumulate.
                for v in range(2):
                    t = 2 * u + v
                    for j in range(4):
                        nc.tensor.matmul(
                            acc[:, j * 65 : (j + 1) * 65],
                            pt[:, v * 512 + j * 128 : v * 512 + (j + 1) * 128],
                            vt[:, t * 65 : (t + 1) * 65],
                            start=(t == 0 and j == 0),
                            stop=(t == KT - 1 and j == 3),
                            skip_group_check=True,
                        )
                if u == CPQ - 1:
                    # drain the 4 accumulated [128,65] slots and write out;
                    # Copy lives in the same ACT table as Exp (no table load),
                    # so assign the drain to the less-loaded of ACT/DVE
                    nonlocal act_t
                    obsl = ob[:, qc * 260 : (qc + 1) * 260]
                    if p == PPC - 1 and qc == NQC - 1:
                        # tail: split drain + DMA across both engines/rings
                        nc.scalar.activation(
                            out=obsl[:, 0:130], in_=acc[:, 0:130], func=COPY,
                            scale=1.0,
                        )
                        nc.vector.tensor_copy(
                            out=obsl[:, 130:260], in_=acc[:, 130:260]
                        )
                        nc.sync.dma_start(
                            out=ot_d[p][qc][:, 0:2, :], in_=obsl[:, 0:130]
                        )
                        nc.gpsimd.dma_start(
                            out=ot_d[p][qc][:, 2:4, :], in_=obsl[:, 130:260]
                        )
                        del accs[(p, qc)]
                        return
                    if act_t + ACT_DRAIN_NS <= dve_t + DVE_DRAIN_NS:
                        nc.scalar.activation(
                            out=obsl, in_=acc[:, 0:260], func=COPY, scale=1.0
                        )
                        act_t += ACT_DRAIN_NS
                    else:
                        nc.vector.tensor_copy(out=obsl, in_=acc[:, 0:260])
                        dve_t += DVE_DRAIN_NS
                    del accs[(p, qc)]
                    out_ring[qc % 2].dma_start(out=ot_d[p][qc], in_=obsl)

            for ci, (p, qc, u) in enumerate(chunks):
                if qc == 0 and u == 0:
                    load_pair(p)
                    load_pair(p + 1)  # prefetch next pair during this one
                qt, kt, vt, ob = tiles[p]
                if u == 0:
                    accs[(p, qc)] = ps_a_pool.tile(
                        [128, 512], F32, name="acc", tag="acc"
                    )
                sc = ps_s_pool.tile([128, 1024], F32, tag="s")
                for v in range(2):
                    t = 2 * u + v
                    nc.tensor.matmul(
                        sc[:, v * 512 : (v + 1) * 512],
                        kt[:, t * 128 : (t + 1) * 128],
                        qt[:, qc * 512 : (qc + 1) * 512],
                        start=True,
                        stop=True,
                    )
                pt = pt_pool.tile([128, 1024], BF16, tag="p")
                if act_t + ACT_CHUNK_NS <= dve_t + DVE_CHUNK_NS:
                    nc.scalar.activation(pt[:], sc[:], EXP, scale=SCALE)
                    act_t += ACT_CHUNK_NS
                else:
                    nc.vector.tensor_scalar(
                        out=pt[:].bitcast(I16),
                        in0=sc[:],
                        scalar1=A_EXP,
                        scalar2=B_EXP,
                        op0=MULT,
                        op1=ADD,
                    )
                    dve_t += DVE_CHUNK_NS
                pts[ci] = pt
                if ci >= 3:
                    emit_pv(ci - 3)
            for ci in range(len(chunks) - 3, len(chunks)):
                emit_pv(ci)
    nc.compile()
    return nc


def _prep_inputs(query, key, value):
    """Host-side layout prep. Returns per-core input maps."""
    q = query.reshape(PAIRS, S, D)
    k = key.reshape(PAIRS, S, D)
    v = value.reshape(PAIRS, S, D)

    qt = np.ascontiguousarray(q.transpose(0, 2, 1)).astype(ml_dtypes.bfloat16)
    kt = np.ascontiguousarray(k.transpose(0, 2, 1)).astype(ml_dtypes.bfloat16)

    vt = v.reshape(PAIRS, KT, 128, D).transpose(0, 2, 1, 3)  # [PAIRS,128,KT,64]
    vo = np.empty((PAIRS, 128, KT, 65), dtype=ml_dtypes.bfloat16)
    vo[:, :, :, :D] = vt.astype(ml_dtypes.bfloat16)
    vo[:, :, :, D] = 1.0
    vo = vo.reshape(PAIRS, 128, KT * 65)

    in_maps = []
    for c in range(N_CORES):
        sl = slice(c * PPC, (c + 1) * PPC)
        in_maps.append(
            {
                "qt": np.ascontiguousarray(qt[sl]),
                "kt": np.ascontiguousarray(kt[sl]),
                "vt": np.ascontiguousarray(vo[sl]),
            }
        )
    return in_maps


_CACHED_NC = None


def kernel(query, key, value, _want_results_obj=False, _trace=False):
    global _CACHED_NC
    if _CACHED_NC is None:
        _CACHED_NC = build_bass()
    nc = _CACHED_NC

    in_maps = _prep_inputs(query, key, value)
    res = run_bass_kernel_spmd(
        nc, in_maps, core_ids=list(range(N_CORES)), trace=_trace
    )

    # per core: [PPC, 4, 128, 4, 65] with rows (qc, q, j) -> (qc, j, q)
    ot = np.concatenate([res.results[c]["ot"] for c in range(N_CORES)], axis=0)
    ot = ot.transpose(0, 1, 3, 2, 4).reshape(PAIRS, S, 65)
    out = ot[:, :, :D] / ot[:, :, D : D + 1]
    out = out.reshape(B, H, S, D).astype(np.float32)
    if _want_results_obj:
        return out, res
    return out


if __name__ == "__main__":
    rng = np.random.default_rng(0)
    q = rng.standard_normal((B, H, S, D), dtype=np.float32)
    k = rng.standard_normal((B, H, S, D), dtype=np.float32)
    v = rng.standard_normal((B, H, S, D), dtype=np.float32)
    o = kernel(query=q, key=k, value=v)
    print("out shape:", o.shape, o.dtype)


# revision 13
# speedup vs baseline: 1.4887x; 1.0115x over previous
"""Trainium2 Bass kernel: batched multi-head attention.

out[b,h] = softmax(Q[b,h] @ K[b,h].T / sqrt(D)) @ V[b,h]
with B=4, H=16, S=2048, D=64, fp32 in/out.

Sharding: the 64 (b,h) pairs are split across 8 NeuronCores, 8 pairs per
core; attention is independent per pair, so no cross-core communication.

Design (v2) — dual-engine softmax + full-utilization PV:

  Under the CoreSim cost model a matmul costs out_free_cols x 0.4167ns
  regardless of K/M, and exp on the scalar engine costs 0.833ns/elem/lane.
  The baseline was ACT-bound (~250us busy). This version:

  1. PV reshape: out[q=128, 65] = pt[k=128, q=128].T @ [V|1][k=128, 65]
     per (q-tile, k-tile) in bf16 (1 cycle/row at any N). PV drops from
     109us to 55.5us; PE total = scores 109.2 + PV 55.5 = 165us busy.
     The ones column makes column 64 the softmax denominator.
  2. exp is split between ACT (exact, exp -> bf16) and DVE (Schraudolph:
     one tensor_scalar computing i16 = rint(x*(128 log2 e)/8 + B) written
     through an int16-bitcast view of the bf16 pt tile — the int16 bit
     pattern IS bf16 2^u, a ~1.8% rms approximation of exp). Greedy
     interleave keeps both engines at ~149us < PE.
  3. Scores stay fp32r (exact): sc[k=128, q=512] = K^T-tile.T @ Q^T-chunk,
     two 512-slices per PSUM chunk [128,1024]; pipeline: scores(c) ->
     exp(c) -> PV(c-2), sc triple-buffered (6 banks) + 2 accumulator
     banks (4 sub-bank [128,65] slots each) = 8 PSUM banks exactly.
  4. Q/K/V are pre-laid host-side in bf16 (halves DMA); inputs ride the
     SP HWDGE ring (qt, kt) and SWDGE (vt); outputs [128,260] drain
     PSUM->SBUF on DVE and DMA out per (pair, qc), alternating SP/SWDGE.
     Output rows are (qc, j, q) interleaved; host undoes it and divides
     by the denominator column.

CoreSim cost model: ~170us/core e2e (PE-bound). Measured HW (PJRT)
relative error vs fp32 reference: ~1.2e-2 (Schraudolph share ~0.42).
"""

import sys

sys.path.insert(0, "/opt/trn_rl_repo")

import numpy as np
import ml_dtypes

import concourse.bacc as bacc
import concourse.mybir as mybir
from concourse.bass_utils import run_bass_kernel_spmd
from concourse.tile import TileContext

B, H, S, D = 4, 16, 2048, 64
N_CORES = 8
PAIRS = B * H              # 64 independent (b, h) attention problems
PPC = PAIRS // N_CORES     # 8 pairs per core
KT = S // 128              # 16 k-tiles of 128 rows
NQC = 4                    # 4 q-chunks of 512
CPQ = 8                    # chunks per (pair, qc): [128, 1024] = 2 score slices
F32 = mybir.dt.float32
F32R = mybir.dt.float32r
BF16 = mybir.dt.bfloat16
I16 = mybir.dt.int16
EXP = mybir.ActivationFunctionType.Exp
COPY = mybir.ActivationFunctionType.Copy
MULT = mybir.AluOpType.mult
ADD = mybir.AluOpType.add
SCALE = 1.0 / np.sqrt(D)

# Schraudolph bf16 exp2 constants (round-to-nearest f32->i16 on DVE):
# i16 = rint(x*SCALE * 128/ln2 + (127*128 - 7.35)); bitcast i16 -> bf16
# approximates exp(x*SCALE) with ~1.78% rms, ~0 mean error.
A_EXP = float(128.0 / np.log(2.0)) * SCALE
B_EXP = 127.0 * 128.0 - 7.35

# Cost-model constants used only to balance the ACT/DVE greedy split.
ACT_CHUNK_NS = 1024 * 0.8333 + 185
DVE_CHUNK_NS = 1024 * 1.0417 + 125
DVE_DRAIN_NS = 260 * 1.0417 + 125
ACT_DRAIN_NS = 260 * 0.8333 + 185
ACT_TABLE_NS = 1283


def build_bass():
    nc = bacc.Bacc()
    qt_d = nc.declare_dram_parameter("qt", [PPC, 64, S], BF16, isOutput=False)
    kt_d = nc.declare_dram_parameter("kt", [PPC, 64, S], BF16, isOutput=False)
    vt_d = nc.declare_dram_parameter("vt", [PPC, 128, KT * 65], BF16, isOutput=False)
    # output rows interleaved (qc, q, j): host reorders + divides by col 64
    ot_d = nc.declare_dram_parameter("ot", [PPC, NQC, 128, 4, 65], F32, isOutput=True)

    with TileContext(nc) as tc:
        with (
            tc.tile_pool(name="qt", bufs=2) as qt_pool,
            tc.tile_pool(name="kt", bufs=2) as kt_pool,
            tc.tile_pool(name="vt", bufs=2) as vt_pool,
            tc.tile_pool(name="pt", bufs=6) as pt_pool,
            tc.tile_pool(name="ob", bufs=2) as ob_pool,
            tc.tile_pool(name="ps_s", bufs=3, space="PSUM") as ps_s_pool,
            tc.tile_pool(name="ps_a", bufs=2, space="PSUM") as ps_a_pool,
        ):
            chunks = [
                (p, qc, u) for p in range(PPC) for qc in range(NQC) for u in range(CPQ)
            ]
            tiles = {}    # pair -> (qt, kt, vt, ob)
            accs = {}     # (pair, qc) -> [128, 512] psum accumulator (4 slots)
            pts = {}      # chunk idx -> pt tile
            act_t, dve_t = ACT_TABLE_NS, 0.0
            last_eng = [None]
            out_ring = [nc.sync, nc.gpsimd]

            def load_pair(p):
                if p in tiles or p >= PPC:
                    return
                qt = qt_pool.tile([64, S], BF16)
                kt = kt_pool.tile([64, S], BF16)
                vt = vt_pool.tile([128, KT * 65], BF16)
                if p == 0:
                    # critical pieces first, on parallel rings, so the first
                    # scores matmul starts ~1.7us in instead of ~4.5us
                    nc.sync.dma_start(out=kt[:, 0:256], in_=kt_d[p][:, 0:256])
                    nc.scalar.dma_start(out=qt[:, 0:512], in_=qt_d[p][:, 0:512])
                    nc.gpsimd.dma_start(out=vt[:], in_=vt_d[p])
                    nc.sync.dma_start(out=kt[:, 256:1280], in_=kt_d[p][:, 256:1280])
                    nc.sync.dma_start(out=kt[:, 1280:S], in_=kt_d[p][:, 1280:S])
                    nc.gpsimd.dma_start(out=qt[:, 512:S], in_=qt_d[p][:, 512:S])
                else:
                    nc.sync.dma_start(out=qt[:], in_=qt_d[p])
                    nc.sync.dma_start(out=kt[:], in_=kt_d[p])
                    nc.gpsimd.dma_start(out=vt[:], in_=vt_d[p])
                ob = ob_pool.tile([128, NQC * 260], F32)
                tiles[p] = (qt, kt, vt, ob)

            def emit_pv(ci):
                nonlocal dve_t
                p, qc, u = chunks[ci]
                pt = pts.pop(ci)
                _, _, vt, ob = tiles[p]
                acc = accs[(p, qc)]
                # start=True marks the whole 2KB PSUM zero-region (bank) as
                # pending-zero, so it must be issued exactly ONCE per bank
                # accumulation round: slots j=1..3's first writes clear their
                # own pending bytes (replace), later writes accumulate.
                for v in range(2):
                    t = 2 * u + v
                    for j in range(4):
                        nc.tensor.matmul(
                            acc[:, j * 65 : (j + 1) * 65],
                            pt[:, v * 512 + j * 128 : v * 512 + (j + 1) * 128],
                            vt[:, t * 65 : (t + 1) * 65],
                            start=(t == 0 and j == 0),
                            stop=(t == KT - 1 and j == 3),
                            skip_group_check=True,
                        )
                if u == CPQ - 1:
                    # drain the 4 accumulated [128,65] slots and write out;
                    # Copy lives in the same ACT table as Exp (no table load),
                    # so assign the drain to the less-loaded of ACT/DVE
                    nonlocal act_t
                    obsl = ob[:, qc * 260 : (qc + 1) * 260]
                    if p == PPC - 1 and qc == NQC - 1:
                        # tail: split drain + DMA across both engines/rings
                        nc.scalar.activation(
                            out=obsl[:, 0:130], in_=acc[:, 0:130], func=COPY,
                            scale=1.0,
                        )
                        nc.vector.tensor_copy(
                            out=obsl[:, 130:260], in_=acc[:, 130:260]
                        )
                        nc.sync.dma_start(
                            out=ot_d[p][qc][:, 0:2, :], in_=obsl[:, 0:130]
                        )
                        nc.gpsimd.dma_start(
                            out=ot_d[p][qc][:, 2:4, :], in_=obsl[:, 130:260]
                        )
                        del accs[(p, qc)]
                        return
                    if act_t + ACT_DRAIN_NS <= dve_t + DVE_DRAIN_NS:
                        nc.scalar.activation(
                            out=obsl, in_=acc[:, 0:260], func=COPY, scale=1.0
                        )
                        act_t += ACT_DRAIN_NS
                    else:
                        nc.vector.tensor_copy(out=obsl, in_=acc[:, 0:260])
                        dve_t += DVE_DRAIN_NS
                    del accs[(p, qc)]
                    out_ring[qc % 2].dma_start(out=ot_d[p][qc], in_=obsl)

            for ci, (p, qc, u) in enumerate(chunks):
                if qc == 0 and u == 0:
                    load_pair(p)
                    load_pair(p + 1)  # prefetch next pair during this one
                qt, kt, vt, ob = tiles[p]
                if u == 0:
                    accs[(p, qc)] = ps_a_pool.tile(
                        [128, 512], F32, name="acc", tag="acc"
                    )
                sc = ps_s_pool.tile([128, 1024], F32, tag="s")
                for v in range(2):
                    t = 2 * u + v
                    nc.tensor.matmul(
                        sc[:, v * 512 : (v + 1) * 512],
                        kt[:, t * 128 : (t + 1) * 128],
                        qt[:, qc * 512 : (qc + 1) * 512],
                        start=True,
                        stop=True,
                    )
                pt = pt_pool.tile([128, 1024], BF16, tag="p")
                if ci >= len(chunks) - 2:
                    # tail: halve exp latency by splitting across both engines
                    nc.scalar.activation(
                        pt[:, 0:512], sc[:, 0:512], EXP, scale=SCALE
                    )
                    nc.vector.tensor_scalar(
                        out=pt[:, 512:1024].bitcast(I16),
                        in0=sc[:, 512:1024],
                        scalar1=A_EXP,
                        scalar2=B_EXP,
                        op0=MULT,
                        op1=ADD,
                    )
                elif act_t + ACT_CHUNK_NS <= dve_t + DVE_CHUNK_NS or (
                    last_eng[0] == "D" and act_t + 2 * ACT_CHUNK_NS
                    <= dve_t + DVE_CHUNK_NS + ACT_CHUNK_NS
                ):
                    nc.scalar.activation(pt[:], sc[:], EXP, scale=SCALE)
                    act_t += ACT_CHUNK_NS
                    last_eng[0] = "A"
                else:
                    nc.vector.tensor_scalar(
                        out=pt[:].bitcast(I16),
                        in0=sc[:],
                        scalar1=A_EXP,
                        scalar2=B_EXP,
                        op0=MULT,
                        op1=ADD,
                    )
                    dve_t += DVE_CHUNK_NS
                    last_eng[0] = "D" 
                pts[ci] = pt
                if ci >= 4:
                    emit_pv(ci - 4)
            for ci in range(len(chunks) - 4, len(chunks)):
                emit_pv(ci)
    nc.compile()
    return nc


def _prep_inputs(query, key, value):
    """Host-side layout prep. Returns per-core input maps."""
    q = query.reshape(PAIRS, S, D)
    k = key.reshape(PAIRS, S, D)
    v = value.reshape(PAIRS, S, D)

    qt = np.ascontiguousarray(q.transpose(0, 2, 1)).astype(ml_dtypes.bfloat16)
    kt = np.ascontiguousarray(k.transpose(0, 2, 1)).astype(ml_dtypes.bfloat16)

    vt = v.reshape(PAIRS, KT, 128, D).transpose(0, 2, 1, 3)  # [PAIRS,128,KT,64]
    vo = np.empty((PAIRS, 128, KT, 65), dtype=ml_dtypes.bfloat16)
    vo[:, :, :, :D] = vt.astype(ml_dtypes.bfloat16)
    vo[:, :, :, D] = 1.0
    vo = vo.reshape(PAIRS, 128, KT * 65)

    in_maps = []
    for c in range(N_CORES):
        sl = slice(c * PPC, (c + 1) * PPC)
        in_maps.append(
            {
                "qt": np.ascontiguousarray(qt[sl]),
                "kt": np.ascontiguousarray(kt[sl]),
                "vt": np.ascontiguousarray(vo[sl]),
            }
        )
    return in_maps


_CACHED_NC = None


def kernel(query, key, value, _want_results_obj=False, _trace=False):
    global _CACHED_NC
    if _CACHED_NC is None:
        _CACHED_NC = build_bass()
    nc = _CACHED_NC

    in_maps = _prep_inputs(query, key, value)
    res = run_bass_kernel_spmd(
        nc, in_maps, core_ids=list(range(N_CORES)), trace=_trace
    )

    # per core: [PPC, 4, 128, 4, 65] with rows (qc, q, j) -> (qc, j, q)
    ot = np.concatenate([res.results[c]["ot"] for c in range(N_CORES)], axis=0)
    ot = ot.transpose(0, 1, 3, 2, 4).reshape(PAIRS, S, 65)
    out = ot[:, :, :D] / ot[:, :, D : D + 1]
    out = out.reshape(B, H, S, D).astype(np.float32)
    if _want_results_obj:
        return out, res
    return out


if __name__ == "__main__":
    rng = np.random.default_rng(0)
    q = rng.standard_normal((B, H, S, D), dtype=np.float32)
    k = rng.standard_normal((B, H, S, D), dtype=np.float32)
    v = rng.standard_normal((B, H, S, D), dtype=np.float32)
    o = kernel(query=q, key=k, value=v)
    print("out shape:", o.shape, o.dtype)
